# revision 17
# baseline (speedup 1.0000x reference)
"""Trainium2 Bass kernel: BlockAttnRes forward (v3).

Reference computation (per batch b, position t):
    k[n]   = s[n] / sqrt(mean(s[n]^2) + eps)        n in [0, 9)
    score  = k[n] . w                                (w = queries[layer_idx])
    alpha  = softmax(score over n)
    h[t]   = sum_n alpha[n] * s[n]                   (d = 512)

Distribution: batch dim B=8 -> one batch per NeuronCore, no cross-core
communication.  Per core: T=4096 positions in 16 MACRO-tiles of 2x128.

v3 changes vs v2 (294.7us):
  - diag(alpha) builds and PSUM->SBUF copies move to GpSimd with a
    3-macro-deep software pipeline so the in-order GpSimd queue never
    serializes PE phases (diag(c) is queued BEFORE copies(c-1); the
    v2 gpsimd experiment lost 35us to the PE->stores->diag chain).
  - BN_Z of the 18 per-macro ssq reductions use DVE bn_stats (gives
    sum(x^2) via count*var + count*mean^2 without a separate square
    pass or ACT accumulator read) to balance ACT vs DVE.
  - alpha (= e * 1/sume) is prebuilt on DVE ([P,18] tensor_scalar_mul,
    cheap) since Pool has no scalar-AP operand slots.

Engine budget per macro (target ~14.3us wall):
    DMA   : 13.9us  (6x 6KB-row loads + 2 bf16 stores = the roofline)
    ACT   : (18-BN_Z) Square+accum, Ln+Exp rsq, Exp e       ~14.2us
    DVE   : 18 dot STT+accum, BN_Z bn_stats+post, softmax smalls,
            sume/recip/al                                    ~13.7us
    GpSimd: diag TT (I*al broadcast), 2 PSUM copies, stores   ~7us
    PE    : 18 accumulating fp32r matmuls                    ~11us
"""

import numpy as np

B, T, N, D = 8, 4096, 9, 512
P = 128
EPS = 1e-6
NCORES = 8
JT = 2              # partition-tiles per macro iteration
MACRO = P * JT      # 256 positions per macro

_CACHE = {}


def _build_bass(
    t_len=T,
    diag_engine="vector",   # "gpsimd" | "vector" (gpsimd: SBUF port contention
                            # inflates DVE+DMA actives ~35us - measured twice)
    copies_engine="mixed",   # "mixed" (ACT/DVE split; Pool cannot read PSUM)
    bn_z=0,                 # (j,n) pairs whose ssq comes from DVE bn_stats
    out_bf16=True,
):
    import concourse.bass as bass
    import concourse.tile as tile
    from concourse import bacc, mybir

    f32 = mybir.dt.float32
    f32r = mybir.dt.float32r
    bf16 = mybir.dt.bfloat16
    f16 = mybir.dt.float16
    Alu = mybir.AluOpType
    Act = mybir.ActivationFunctionType
    Ax = mybir.AxisListType

    nmacro = t_len // MACRO
    out_dt = bf16 if out_bf16 else f32

    PINNED_SET = "natural_log_exp_and_others"

    class PinnedBacc(bacc.Bacc):
        def insert_act_table_loads(self):
            import bass_rust as _bass_rust
            from concourse.hw_specs import get_activation_tables

            all_tables = get_activation_tables(self.m.arch)
            used = {
                i.func
                for b in self.main_func.blocks
                for i in b.instructions
                if isinstance(i, mybir.InstActivation)
            }
            if used and PINNED_SET in all_tables and used <= all_tables[PINNED_SET]:
                tables = [
                    (name, funcs if name == PINNED_SET else set())
                    for name, funcs in all_tables.items()
                ]
            else:
                tables = list(all_tables.items())
            _bass_rust.insert_act_table_loads(self, tables)

    nc = PinnedBacc("TRN2", target_bir_lowering=False, debug=False)
    src = nc.dram_tensor("src", [t_len, N, D], f32, kind="ExternalInput").ap()
    # w in fp16: halves the per-STT w re-read SBUF traffic (75MB total);
    # products still accumulate in f32 (measured 5.6e-7 rel err).
    wq = nc.dram_tensor("wq", [P, D], f16, kind="ExternalInput").ap()
    idn = nc.dram_tensor("idn", [P, P], f32, kind="ExternalInput").ap()
    out = nc.dram_tensor("out", [t_len, D], out_dt, kind="ExternalOutput").ap()

    src_t = src.rearrange("(c j p) n d -> c j p n d", j=JT, p=P)
    out_t = out.rearrange("(c j p) d -> c j p d", j=JT, p=P)

    NCH = 3           # n-chunks per partition-tile
    CN = N // NCH     # n's per chunk

    # (j, n) pairs whose ssq comes from bn_stats on DVE (off the ACT queue).
    bn_pairs = [(1, N - 1 - i) for i in range(bn_z)]

    def bc(ap, reps):
        """Insert a stride-0 dim after the partition dim."""
        return bass.AP(
            tensor=ap.tensor,
            offset=ap.offset,
            ap=[ap.ap[0], [0, reps], *ap.ap[1:]],
        )

    def bc_inner(ap, reps):
        """Append a stride-0 innermost dim."""
        return bass.AP(
            tensor=ap.tensor,
            offset=ap.offset,
            ap=[*ap.ap, [0, reps]],
        )

    with tile.TileContext(nc) as tc:
        with (
            tc.tile_pool(name="const", bufs=1) as const_pool,
            tc.tile_pool(name="srcp", bufs=4) as src_pool,
            tc.tile_pool(name="scratch", bufs=3) as scr_pool,
            tc.tile_pool(name="small", bufs=7) as small_pool,
            tc.tile_pool(name="diag", bufs=2) as diag_pool,
            tc.tile_pool(name="hout", bufs=4) as out_pool,
            tc.tile_pool(name="psum", bufs=4, space="PSUM") as psum_pool,
        ):
            state = {}
            const_tiles = {}

            def emit_w():
                # w gates the first dot STTs -> its DMA goes first of all.
                w_sb = const_pool.tile([P, D], f16, name="w_sb")
                nc.sync.dma_start(out=w_sb, in_=wq)
                eps_sb = const_pool.tile([P, 1], f32, name="eps_sb")
                nc.vector.memset(eps_sb, EPS)
                const_tiles.update(w=w_sb, eps=eps_sb)

            def emit_idn():
                i_sb = const_pool.tile([P, P], f32, name="i_sb")
                nc.sync.dma_start(out=i_sb, in_=idn)
                const_tiles.update(i=i_sb)

            def emit_loads(c):
                # Tiles are float32r-typed (verifier demands fp32r matmul
                # inputs come from fp32r locations); DVE/ACT consumers read
                # them bitcast back to f32 - same bytes.
                chunks = [[None] * NCH for _ in range(JT)]
                for j in range(JT):
                    for k in range(NCH):
                        sk = src_pool.tile([P, CN, D], f32r, tag=f"s{j}{k}")
                        # prologue: j1 triggers ride the idle GpSimd SWDGE
                        # queue so the Sync queue isn't the issue bottleneck
                        # during ramp-up.
                        eng = nc.gpsimd if (c < 2 and j == 1) else nc.sync
                        eng.dma_start(
                            out=sk,
                            in_=src_t[c, j, :, k * CN : (k + 1) * CN, :].bitcast(f32r),
                        )
                        chunks[j][k] = sk
                state[c] = {"chunks": chunks}

            def s_mm(c, j, n):
                return state[c]["chunks"][j][n // CN][:, n % CN, :]

            def s_of(c, j, n):
                return s_mm(c, j, n).bitcast(f32)

            def emit_passes(c):
                """Bulk streaming: dots (DVE), ssq (ACT squares + DVE
                bn_stats for bn_pairs), rsq (ACT)."""
                st = state[c]
                # fp16 scratch outs: the main outs of the dot/square streams
                # are discarded; fp16 halves their SBUF write traffic
                # (151MB total) to relieve the shared SBUF port.
                dot = small_pool.tile([P, JT, N], f32, tag="dot")
                prod = scr_pool.tile([P, D], f16, tag="prod")
                for j in range(JT):
                    for n in range(N):
                        nc.vector.scalar_tensor_tensor(
                            out=prod,
                            in0=s_of(c, j, n),
                            scalar=0.0,
                            in1=const_tiles["w"],
                            op0=Alu.bypass,
                            op1=Alu.mult,
                            accum_out=dot[:, j, n : n + 1],
                        )
                ssq = small_pool.tile([P, JT, N], f32, tag="ssq")
                sq = scr_pool.tile([P, D], f16, tag="sq")
                for j in range(JT):
                    for n in range(N):
                        if (j, n) in bn_pairs:
                            continue
                        nc.scalar.activation(
                            out=sq,
                            in_=s_of(c, j, n),
                            func=Act.Square,
                            accum_out=ssq[:, j, n : n + 1],
                        )
                if bn_pairs:
                    # bn_stats -> [P, z, 6]: per group
                    # [c_e, m_e, c_e*var_e, c_o, m_o, c_o*var_o];
                    # ssq = (cv_e + cv_o) + 256*(m_e^2 + m_o^2)
                    z = len(bn_pairs)
                    bnt = small_pool.tile([P, z, 6], f32, tag="bnt")
                    for i, (j, n) in enumerate(bn_pairs):
                        nc.vector.bn_stats(out=bnt[:, i, :], in_=s_of(c, j, n))
                    m2e = small_pool.tile([P, z], f32, tag="m2e")
                    nc.vector.tensor_mul(m2e, bnt[:, :, 1], bnt[:, :, 1])
                    m2o = small_pool.tile([P, z], f32, tag="m2o")
                    nc.vector.tensor_mul(m2o, bnt[:, :, 4], bnt[:, :, 4])
                    m2 = small_pool.tile([P, z], f32, tag="m2")
                    nc.vector.tensor_add(m2, m2e, m2o)
                    cv = small_pool.tile([P, z], f32, tag="cv")
                    nc.vector.tensor_add(cv, bnt[:, :, 2], bnt[:, :, 5])
                    # ssq slice: bn_pairs are (1, N-1), (1, N-2), ... ->
                    # write each individually (reversed order, noncontig ok)
                    for i, (j, n) in enumerate(bn_pairs):
                        nc.vector.scalar_tensor_tensor(
                            out=ssq[:, j, n : n + 1],
                            in0=m2[:, i : i + 1],
                            scalar=float(D // 2),
                            in1=cv[:, i : i + 1],
                            op0=Alu.mult,
                            op1=Alu.add,
                        )
                # rsq = (ssq/D + eps)^(-1/2) via Exp(-0.5*Ln(x))
                rsq = small_pool.tile([P, JT, N], f32, tag="rsq")
                nc.scalar.activation(
                    out=rsq,
                    in_=ssq,
                    func=Act.Ln,
                    scale=1.0 / D,
                    bias=const_tiles["eps"],
                )
                nc.scalar.activation(out=rsq, in_=rsq, func=Act.Exp, scale=-0.5)
                st["dot"], st["rsq"] = dot, rsq

            def emit_front(c):
                """score + (negated) row max + max-subtract on DVE."""
                st = state[c]
                score = small_pool.tile([P, JT, N], f32, tag="score")
                nc.vector.tensor_mul(score, st["dot"], st["rsq"])
                nmx = small_pool.tile([P, JT], f32, tag="nmx")
                nc.vector.tensor_reduce(
                    out=nmx, in_=score, axis=Ax.X, op=Alu.max, negate=True
                )
                score2 = small_pool.tile([P, JT, N], f32, tag="score2")
                nc.vector.tensor_add(score2, score, bc_inner(nmx, N))
                st["score2"] = score2

            def emit_exp(c):
                """e = exp(score - max) on ACT (one instr, both j)."""
                st = state[c]
                e = small_pool.tile([P, JT, N], f32, tag="e")
                nc.scalar.activation(out=e, in_=st["score2"], func=Act.Exp)
                st["e"] = e

            def emit_small(c):
                """sume/recip/alpha on DVE (first in DVE queue of iter c:
                e(c) lands at the end of iter c-1, so no queue-head stall)."""
                st = state[c]
                e = st["e"]
                sume = small_pool.tile([P, JT], f32, tag="sume")
                nc.vector.tensor_reduce(out=sume, in_=e, axis=Ax.X, op=Alu.add)
                rs = small_pool.tile([P, JT], f32, tag="rs")
                nc.vector.reciprocal(out=rs, in_=sume)
                if diag_engine == "gpsimd":
                    al = small_pool.tile([P, JT * N], f32, tag="al")
                    for j in range(JT):
                        nc.vector.tensor_scalar_mul(
                            al[:, j * N : (j + 1) * N], e[:, j, :], rs[:, j : j + 1]
                        )
                    st["al"] = al
                st["rs"] = rs

            def emit_diag(c):
                """diag(alpha) build: one GpSimd TT over [P, 18, P]."""
                st = state[c]
                dg = diag_pool.tile([P, JT * N, P], f32r, tag="dg")
                if diag_engine == "gpsimd":
                    nc.gpsimd.tensor_tensor(
                        out=dg,
                        in0=bc(const_tiles["i"], JT * N),
                        in1=bc_inner(st["al"], P),
                        op=Alu.mult,
                    )
                else:
                    e, rs = st["e"], st["rs"]
                    for j in range(JT):
                        nc.vector.scalar_tensor_tensor(
                            out=dg[:, j * N : (j + 1) * N, :],
                            in0=bc(const_tiles["i"], N),
                            scalar=rs[:, j : j + 1],
                            in1=bc_inner(e[:, j, :], P),
                            op0=Alu.mult,
                            op1=Alu.mult,
                        )
                st["dg"] = dg

            def emit_mm(c, jset=None):
                """the fp32r accumulating matmuls (optionally one j)."""
                st = state[c]
                dg = st["dg"]
                hps = st.setdefault("hps", {})
                for j in jset if jset is not None else range(JT):
                    hp = psum_pool.tile([P, D], f32, tag=f"hp{j}")
                    for n in range(N):
                        nc.tensor.matmul(
                            hp,
                            dg[:, j * N + n, :],
                            s_mm(c, j, n),
                            start=(n == 0),
                            stop=(n == N - 1),
                        )
                    hps[j] = hp

            def emit_copies(c, jset=None):
                """PSUM -> SBUF (+ bf16 cast) + stores.

                Last macro: copies on ACT (idle during drain) + stores on
                the HWDGE queue (empty by then)."""
                st = state[c]
                if "hs" not in st:
                    hs = out_pool.tile([P, JT, D], out_dt, tag="hs")
                    st["hs"] = hs
                hs = st["hs"]
                last = c == nmacro - 1
                for j in jset if jset is not None else range(JT):
                    hp = st["hps"][j]
                    if last or copies_engine != "gpsimd":
                        if last or (j == 0 and c % 3 != 0):
                            nc.scalar.activation(
                                out=hs[:, j, :], in_=hp, func=Act.Copy
                            )
                        else:
                            nc.vector.tensor_copy(out=hs[:, j, :], in_=hp)
                    else:
                        nc.gpsimd.tensor_copy(out=hs[:, j, :], in_=hp)
                    if last:
                        nc.sync.dma_start(out=out_t[c, j], in_=hs[:, j, :])
                    else:
                        nc.gpsimd.dma_start(out=out_t[c, j], in_=hs[:, j, :])
                if (jset is None) or (JT - 1 in jset):
                    del state[c]

            # Deep software pipeline. Iteration c queues:
            #   DVE: sume/recip/al(c), dots(c+2)+bn(c+2), score/nmx/sub(c+1)
            #   ACT: squares(c+2)+Ln/Exp(c+2), exp(c+1)
            #   GpS: diag(c), copies(c-1)+stores(c-1)
            #   PE : matmuls(c)
            #   Sync: loads(c+3)
            emit_w()
            emit_loads(0)
            emit_loads(1)
            emit_idn()
            emit_passes(0)
            emit_loads(2)
            emit_passes(1)
            emit_front(0)
            emit_exp(0)
            LAST = nmacro - 1
            for c in range(nmacro):
                if c < LAST:
                    emit_small(c)
                    emit_diag(c)
                if c + 3 < nmacro:
                    emit_loads(c + 3)
                if c + 2 < nmacro:
                    emit_passes(c + 2)
                if c < LAST:
                    emit_mm(c)
                if c + 1 < nmacro:
                    emit_front(c + 1)
                    emit_exp(c + 1)
                if c == LAST - 1:
                    # tail compression: queue the last macro's softmax/diag
                    # right behind exp(LAST) so its matmuls can overlap the
                    # preceding macro's drain (small one-time DVE wait).
                    emit_small(LAST)
                    emit_diag(LAST)
                if c >= 1:
                    emit_copies(c - 1)
            # last macro: interleave per-j matmul+copy+store so j0's chain
            # drains while PE runs j1.
            emit_mm(LAST, jset=[0])
            emit_copies(LAST, jset=[0])
            emit_mm(LAST, jset=[1])
            emit_copies(LAST, jset=[1])

    nc.compile()
    return nc


def _get_nc(t_len=T, **kw):
    key = (t_len, tuple(sorted(kw.items())))
    if key not in _CACHE:
        _CACHE[key] = _build_bass(t_len, **kw)
    return _CACHE[key]


def _make_in_maps(sources, queries, layer_idx):
    sources = np.ascontiguousarray(np.asarray(sources, dtype=np.float32))
    queries = np.asarray(queries, dtype=np.float32)
    w = queries[int(layer_idx)]
    w_rep = np.ascontiguousarray(np.broadcast_to(w[None, :], (P, D)).astype(np.float16))
    idn = np.eye(P, dtype=np.float32)
    return [
        {"src": np.ascontiguousarray(sources[b]), "wq": w_rep, "idn": idn}
        for b in range(sources.shape[0])
    ]


def kernel(sources, queries, layer_idx):
    from concourse.bass_utils import run_bass_kernel_spmd

    nc = _get_nc()
    in_maps = _make_in_maps(sources, queries, layer_idx)
    res = run_bass_kernel_spmd(nc, in_maps, core_ids=list(range(NCORES)))
    outs = [
        np.asarray(res.results[b]["out"]).astype(np.float32) for b in range(NCORES)
    ]
    return np.stack(outs, axis=0)


# revision 18
# speedup vs baseline: 1.0366x; 1.0366x over previous
"""Trainium2 Bass kernel: BlockAttnRes forward (v3).

Reference computation (per batch b, position t):
    k[n]   = s[n] / sqrt(mean(s[n]^2) + eps)        n in [0, 9)
    score  = k[n] . w                                (w = queries[layer_idx])
    alpha  = softmax(score over n)
    h[t]   = sum_n alpha[n] * s[n]                   (d = 512)

Distribution: batch dim B=8 -> one batch per NeuronCore, no cross-core
communication.  Per core: T=4096 positions in 16 MACRO-tiles of 2x128.

v3 changes vs v2 (294.7us):
  - diag(alpha) builds and PSUM->SBUF copies move to GpSimd with a
    3-macro-deep software pipeline so the in-order GpSimd queue never
    serializes PE phases (diag(c) is queued BEFORE copies(c-1); the
    v2 gpsimd experiment lost 35us to the PE->stores->diag chain).
  - BN_Z of the 18 per-macro ssq reductions use DVE bn_stats (gives
    sum(x^2) via count*var + count*mean^2 without a separate square
    pass or ACT accumulator read) to balance ACT vs DVE.
  - alpha (= e * 1/sume) is prebuilt on DVE ([P,18] tensor_scalar_mul,
    cheap) since Pool has no scalar-AP operand slots.

Engine budget per macro (target ~14.3us wall):
    DMA   : 13.9us  (6x 6KB-row loads + 2 bf16 stores = the roofline)
    ACT   : (18-BN_Z) Square+accum, Ln+Exp rsq, Exp e       ~14.2us
    DVE   : 18 dot STT+accum, BN_Z bn_stats+post, softmax smalls,
            sume/recip/al                                    ~13.7us
    GpSimd: diag TT (I*al broadcast), 2 PSUM copies, stores   ~7us
    PE    : 18 accumulating fp32r matmuls                    ~11us
"""

import numpy as np

B, T, N, D = 8, 4096, 9, 512
P = 128
EPS = 1e-6
NCORES = 8
JT = 2              # partition-tiles per macro iteration
MACRO = P * JT      # 256 positions per macro

_CACHE = {}


def _build_bass(
    t_len=T,
    diag_engine="vector",   # "gpsimd" | "vector" (gpsimd: SBUF port contention
                            # inflates DVE+DMA actives ~35us - measured twice)
    copies_engine="mixed",   # "mixed" (ACT/DVE split; Pool cannot read PSUM)
    bn_z=0,                 # (j,n) pairs whose ssq comes from DVE bn_stats
    out_bf16=True,
):
    import concourse.bass as bass
    import concourse.tile as tile
    from concourse import bacc, mybir

    f32 = mybir.dt.float32
    f32r = mybir.dt.float32r
    bf16 = mybir.dt.bfloat16
    f16 = mybir.dt.float16
    Alu = mybir.AluOpType
    Act = mybir.ActivationFunctionType
    Ax = mybir.AxisListType

    nmacro = t_len // MACRO
    out_dt = bf16 if out_bf16 else f32

    PINNED_SET = "natural_log_exp_and_others"

    class PinnedBacc(bacc.Bacc):
        def insert_act_table_loads(self):
            import bass_rust as _bass_rust
            from concourse.hw_specs import get_activation_tables

            all_tables = get_activation_tables(self.m.arch)
            used = {
                i.func
                for b in self.main_func.blocks
                for i in b.instructions
                if isinstance(i, mybir.InstActivation)
            }
            if used and PINNED_SET in all_tables and used <= all_tables[PINNED_SET]:
                tables = [
                    (name, funcs if name == PINNED_SET else set())
                    for name, funcs in all_tables.items()
                ]
            else:
                tables = list(all_tables.items())
            _bass_rust.insert_act_table_loads(self, tables)

    nc = PinnedBacc("TRN2", target_bir_lowering=False, debug=False)
    src = nc.dram_tensor("src", [t_len, N, D], f32, kind="ExternalInput").ap()
    # w in fp16: halves the per-STT w re-read SBUF traffic (75MB total);
    # products still accumulate in f32 (measured 5.6e-7 rel err).
    wq = nc.dram_tensor("wq", [P, D], f16, kind="ExternalInput").ap()
    idn = nc.dram_tensor("idn", [P, P], f32, kind="ExternalInput").ap()
    out = nc.dram_tensor("out", [t_len, D], out_dt, kind="ExternalOutput").ap()

    src_t = src.rearrange("(c j p) n d -> c j p n d", j=JT, p=P)
    out_t = out.rearrange("(c j p) d -> c j p d", j=JT, p=P)

    NCH = 3           # n-chunks per partition-tile
    CN = N // NCH     # n's per chunk

    # (j, n) pairs whose ssq comes from bn_stats on DVE (off the ACT queue).
    bn_pairs = [(1, N - 1 - i) for i in range(bn_z)]

    def bc(ap, reps):
        """Insert a stride-0 dim after the partition dim."""
        return bass.AP(
            tensor=ap.tensor,
            offset=ap.offset,
            ap=[ap.ap[0], [0, reps], *ap.ap[1:]],
        )

    def bc_inner(ap, reps):
        """Append a stride-0 innermost dim."""
        return bass.AP(
            tensor=ap.tensor,
            offset=ap.offset,
            ap=[*ap.ap, [0, reps]],
        )

    with tile.TileContext(nc) as tc:
        with (
            tc.tile_pool(name="const", bufs=1) as const_pool,
            tc.tile_pool(name="srcp", bufs=4) as src_pool,
            tc.tile_pool(name="scratch", bufs=3) as scr_pool,
            tc.tile_pool(name="small", bufs=7) as small_pool,
            tc.tile_pool(name="diag", bufs=2) as diag_pool,
            tc.tile_pool(name="hout", bufs=4) as out_pool,
            tc.tile_pool(name="psum", bufs=4, space="PSUM") as psum_pool,
        ):
            state = {}
            const_tiles = {}

            def emit_w():
                # w gates the first dot STTs -> its DMA goes first of all.
                w_sb = const_pool.tile([P, D], f16, name="w_sb")
                nc.sync.dma_start(out=w_sb, in_=wq)
                eps_sb = const_pool.tile([P, 1], f32, name="eps_sb")
                nc.vector.memset(eps_sb, EPS)
                const_tiles.update(w=w_sb, eps=eps_sb)

            def emit_idn():
                i_sb = const_pool.tile([P, P], f32, name="i_sb")
                nc.sync.dma_start(out=i_sb, in_=idn)
                const_tiles.update(i=i_sb)

            def emit_loads(c):
                # Tiles are float32r-typed (verifier demands fp32r matmul
                # inputs come from fp32r locations); DVE/ACT consumers read
                # them bitcast back to f32 - same bytes.
                chunks = [[None] * NCH for _ in range(JT)]
                for j in range(JT):
                    for k in range(NCH):
                        sk = src_pool.tile([P, CN, D], f32r, tag=f"s{j}{k}")
                        nc.sync.dma_start(
                            out=sk,
                            in_=src_t[c, j, :, k * CN : (k + 1) * CN, :].bitcast(f32r),
                        )
                        chunks[j][k] = sk
                state[c] = {"chunks": chunks}

            def s_mm(c, j, n):
                return state[c]["chunks"][j][n // CN][:, n % CN, :]

            def s_of(c, j, n):
                return s_mm(c, j, n).bitcast(f32)

            def emit_passes(c):
                """Bulk streaming: dots (DVE), ssq (ACT squares + DVE
                bn_stats for bn_pairs), rsq (ACT)."""
                st = state[c]
                # fp16 scratch outs: the main outs of the dot/square streams
                # are discarded; fp16 halves their SBUF write traffic
                # (151MB total) to relieve the shared SBUF port.
                dot = small_pool.tile([P, JT, N], f32, tag="dot")
                prod = scr_pool.tile([P, D], f16, tag="prod")
                for j in range(JT):
                    for n in range(N):
                        nc.vector.scalar_tensor_tensor(
                            out=prod,
                            in0=s_of(c, j, n),
                            scalar=0.0,
                            in1=const_tiles["w"],
                            op0=Alu.bypass,
                            op1=Alu.mult,
                            accum_out=dot[:, j, n : n + 1],
                        )
                ssq = small_pool.tile([P, JT, N], f32, tag="ssq")
                sq = scr_pool.tile([P, D], f16, tag="sq")
                for j in range(JT):
                    for n in range(N):
                        if (j, n) in bn_pairs:
                            continue
                        nc.scalar.activation(
                            out=sq,
                            in_=s_of(c, j, n),
                            func=Act.Square,
                            accum_out=ssq[:, j, n : n + 1],
                        )
                if bn_pairs:
                    # bn_stats -> [P, z, 6]: per group
                    # [c_e, m_e, c_e*var_e, c_o, m_o, c_o*var_o];
                    # ssq = (cv_e + cv_o) + 256*(m_e^2 + m_o^2)
                    z = len(bn_pairs)
                    bnt = small_pool.tile([P, z, 6], f32, tag="bnt")
                    for i, (j, n) in enumerate(bn_pairs):
                        nc.vector.bn_stats(out=bnt[:, i, :], in_=s_of(c, j, n))
                    m2e = small_pool.tile([P, z], f32, tag="m2e")
                    nc.vector.tensor_mul(m2e, bnt[:, :, 1], bnt[:, :, 1])
                    m2o = small_pool.tile([P, z], f32, tag="m2o")
                    nc.vector.tensor_mul(m2o, bnt[:, :, 4], bnt[:, :, 4])
                    m2 = small_pool.tile([P, z], f32, tag="m2")
                    nc.vector.tensor_add(m2, m2e, m2o)
                    cv = small_pool.tile([P, z], f32, tag="cv")
                    nc.vector.tensor_add(cv, bnt[:, :, 2], bnt[:, :, 5])
                    # ssq slice: bn_pairs are (1, N-1), (1, N-2), ... ->
                    # write each individually (reversed order, noncontig ok)
                    for i, (j, n) in enumerate(bn_pairs):
                        nc.vector.scalar_tensor_tensor(
                            out=ssq[:, j, n : n + 1],
                            in0=m2[:, i : i + 1],
                            scalar=float(D // 2),
                            in1=cv[:, i : i + 1],
                            op0=Alu.mult,
                            op1=Alu.add,
                        )
                # rsq = (ssq/D + eps)^(-1/2) via Exp(-0.5*Ln(x))
                rsq = small_pool.tile([P, JT, N], f32, tag="rsq")
                nc.scalar.activation(
                    out=rsq,
                    in_=ssq,
                    func=Act.Ln,
                    scale=1.0 / D,
                    bias=const_tiles["eps"],
                )
                nc.scalar.activation(out=rsq, in_=rsq, func=Act.Exp, scale=-0.5)
                st["dot"], st["rsq"] = dot, rsq

            def emit_front(c):
                """score + (negated) row max + max-subtract on DVE."""
                st = state[c]
                score = small_pool.tile([P, JT, N], f32, tag="score")
                nc.vector.tensor_mul(score, st["dot"], st["rsq"])
                nmx = small_pool.tile([P, JT], f32, tag="nmx")
                nc.vector.tensor_reduce(
                    out=nmx, in_=score, axis=Ax.X, op=Alu.max, negate=True
                )
                score2 = small_pool.tile([P, JT, N], f32, tag="score2")
                nc.vector.tensor_add(score2, score, bc_inner(nmx, N))
                st["score2"] = score2

            def emit_exp(c):
                """e = exp(score - max) on ACT (one instr, both j)."""
                st = state[c]
                e = small_pool.tile([P, JT, N], f32, tag="e")
                nc.scalar.activation(out=e, in_=st["score2"], func=Act.Exp)
                st["e"] = e

            def emit_small(c):
                """sume/recip/alpha on DVE (first in DVE queue of iter c:
                e(c) lands at the end of iter c-1, so no queue-head stall)."""
                st = state[c]
                e = st["e"]
                sume = small_pool.tile([P, JT], f32, tag="sume")
                nc.vector.tensor_reduce(out=sume, in_=e, axis=Ax.X, op=Alu.add)
                rs = small_pool.tile([P, JT], f32, tag="rs")
                nc.vector.reciprocal(out=rs, in_=sume)
                if diag_engine == "gpsimd":
                    al = small_pool.tile([P, JT * N], f32, tag="al")
                    for j in range(JT):
                        nc.vector.tensor_scalar_mul(
                            al[:, j * N : (j + 1) * N], e[:, j, :], rs[:, j : j + 1]
                        )
                    st["al"] = al
                st["rs"] = rs

            def emit_diag(c):
                """diag(alpha) build: one GpSimd TT over [P, 18, P]."""
                st = state[c]
                dg = diag_pool.tile([P, JT * N, P], f32r, tag="dg")
                if diag_engine == "gpsimd":
                    nc.gpsimd.tensor_tensor(
                        out=dg,
                        in0=bc(const_tiles["i"], JT * N),
                        in1=bc_inner(st["al"], P),
                        op=Alu.mult,
                    )
                else:
                    e, rs = st["e"], st["rs"]
                    for j in range(JT):
                        nc.vector.scalar_tensor_tensor(
                            out=dg[:, j * N : (j + 1) * N, :],
                            in0=bc(const_tiles["i"], N),
                            scalar=rs[:, j : j + 1],
                            in1=bc_inner(e[:, j, :], P),
                            op0=Alu.mult,
                            op1=Alu.mult,
                        )
                st["dg"] = dg

            def emit_mm(c, jset=None):
                """the fp32r accumulating matmuls (optionally one j)."""
                st = state[c]
                dg = st["dg"]
                hps = st.setdefault("hps", {})
                for j in jset if jset is not None else range(JT):
                    hp = psum_pool.tile([P, D], f32, tag=f"hp{j}")
                    for n in range(N):
                        nc.tensor.matmul(
                            hp,
                            dg[:, j * N + n, :],
                            s_mm(c, j, n),
                            start=(n == 0),
                            stop=(n == N - 1),
                        )
                    hps[j] = hp

            def emit_copies(c, jset=None):
                """PSUM -> SBUF (+ bf16 cast) + stores.

                Last macro: copies on ACT (idle during drain) + stores on
                the HWDGE queue (empty by then)."""
                st = state[c]
                if "hs" not in st:
                    hs = out_pool.tile([P, JT, D], out_dt, tag="hs")
                    st["hs"] = hs
                hs = st["hs"]
                last = c == nmacro - 1
                for j in jset if jset is not None else range(JT):
                    hp = st["hps"][j]
                    if last or copies_engine != "gpsimd":
                        if last or (j == 0 and c % 3 != 0):
                            nc.scalar.activation(
                                out=hs[:, j, :], in_=hp, func=Act.Copy
                            )
                        else:
                            nc.vector.tensor_copy(out=hs[:, j, :], in_=hp)
                    else:
                        nc.gpsimd.tensor_copy(out=hs[:, j, :], in_=hp)
                    if last:
                        nc.sync.dma_start(out=out_t[c, j], in_=hs[:, j, :])
                    else:
                        nc.gpsimd.dma_start(out=out_t[c, j], in_=hs[:, j, :])
                if (jset is None) or (JT - 1 in jset):
                    del state[c]

            # Deep software pipeline. Iteration c queues:
            #   DVE: sume/recip/al(c), dots(c+2)+bn(c+2), score/nmx/sub(c+1)
            #   ACT: squares(c+2)+Ln/Exp(c+2), exp(c+1)
            #   GpS: diag(c), copies(c-1)+stores(c-1)
            #   PE : matmuls(c)
            #   Sync: loads(c+3)
            emit_w()
            emit_loads(0)
            emit_loads(1)
            emit_idn()
            emit_passes(0)
            emit_loads(2)
            emit_passes(1)
            emit_front(0)
            emit_exp(0)
            LAST = nmacro - 1
            for c in range(nmacro):
                if c < LAST:
                    emit_small(c)
                    emit_diag(c)
                if c + 3 < nmacro:
                    emit_loads(c + 3)
                if c + 2 < nmacro:
                    emit_passes(c + 2)
                if c < LAST:
                    emit_mm(c)
                if c + 1 < nmacro:
                    emit_front(c + 1)
                    emit_exp(c + 1)
                if c == LAST - 1:
                    # tail compression: queue the last macro's softmax/diag
                    # right behind exp(LAST) so its matmuls can overlap the
                    # preceding macro's drain (small one-time DVE wait).
                    emit_small(LAST)
                    emit_diag(LAST)
                if c >= 1:
                    emit_copies(c - 1)
            # last macro: interleave per-j matmul+copy+store so j0's chain
            # drains while PE runs j1.
            emit_mm(LAST, jset=[0])
            emit_copies(LAST, jset=[0])
            emit_mm(LAST, jset=[1])
            emit_copies(LAST, jset=[1])

    nc.compile()
    return nc


def _get_nc(t_len=T, **kw):
    key = (t_len, tuple(sorted(kw.items())))
    if key not in _CACHE:
        _CACHE[key] = _build_bass(t_len, **kw)
    return _CACHE[key]


def _make_in_maps(sources, queries, layer_idx):
    sources = np.ascontiguousarray(np.asarray(sources, dtype=np.float32))
    queries = np.asarray(queries, dtype=np.float32)
    w = queries[int(layer_idx)]
    w_rep = np.ascontiguousarray(np.broadcast_to(w[None, :], (P, D)).astype(np.float16))
    idn = np.eye(P, dtype=np.float32)
    return [
        {"src": np.ascontiguousarray(sources[b]), "wq": w_rep, "idn": idn}
        for b in range(sources.shape[0])
    ]


def kernel(sources, queries, layer_idx):
    from concourse.bass_utils import run_bass_kernel_spmd

    nc = _get_nc()
    in_maps = _make_in_maps(sources, queries, layer_idx)
    res = run_bass_kernel_spmd(nc, in_maps, core_ids=list(range(NCORES)))
    outs = [
        np.asarray(res.results[b]["out"]).astype(np.float32) for b in range(NCORES)
    ]
    return np.stack(outs, axis=0)
